# revision 1
# baseline (speedup 1.0000x reference)
"""DSA sparse attention (context-parallel variant) for Trainium2 via Bass/Tile.

Dense-rewrite algorithm (mathematically identical to the reference):
  w[s,t] = exp(sc[s,t])*ts[s,t] / sum_t' exp(sc)*ts   (softmax->*ts->renorm collapses)
  TS[s,j] = sum_t ts[s,t]*[idx[s,t]==j]  -> scatter of input values (dup-safe)
  E[s,j]  = TS[s,j]*exp(scale*S[s,j]),  S = Q K^T (dense)
  O       = (E @ V) / rowsum(E)
Everything is computed in transposed layout (kv on partitions); O comes out
natural via E^T-stationary matmuls; rowsum(E) falls out of a ones-column
appended to V.
"""

import sys

sys.path.insert(0, "/opt/trn_rl_repo")

import numpy as np

import concourse.bass as bass
import concourse.bacc as bacc
import concourse.mybir as mybir
import concourse.tile as tile
from concourse import library_config, masks
from concourse.vector_clock import ScopedClock

# ---------------------------------------------------------------------------
# Patch: this walrus build encodes at most ONE sync-wait on a CTRL NO_STRUCT
# instruction; TileContext's tail drain carries one wait per live proc.  Split
# the waits across a chain of single-wait drains.
# ---------------------------------------------------------------------------


def _patched_drain_and_barrier(self, tick_clock, wait_clock):
    drain_inst = self.nc.sync.drain()
    wait_clock.add_sem_waits(
        drain_inst.ins, ScopedClock({None: tick_clock.global_clock})
    )
    si = drain_inst.ins.sync_info
    if si is not None and len(si.on_wait) > 1:
        waits = list(si.on_wait)
        drain_inst.ins.sync_info = mybir.SyncInfo(
            on_wait=waits[:1], on_update=list(si.on_update)
        )
        for i in range(1, len(waits)):
            extra = self.nc.sync.drain()
            extra.ins.sync_info = mybir.SyncInfo(on_wait=[waits[i]], on_update=[])
    self.nc.all_engine_barrier()
    assert self.sems is not None
    popped = self.nc._tile_sem_poison_stack.pop()
    assert popped is self._sem_poison
    self.nc.clear_and_free_semaphores(list(self.sems.allocated().values()))
    self.nc.all_engine_barrier()


tile.TileContext._drain_and_barrier = _patched_drain_and_barrier

FP = mybir.dt.float32
BF = mybir.dt.bfloat16
I16 = mybir.dt.int16


class Cfg:
    def __init__(self, HPC=2, SQ=1024, SKV=4096, D=128, TOPK=64):
        self.HPC = HPC  # heads per core
        self.SQ = SQ
        self.SKV = SKV
        self.D = D
        self.TOPK = TOPK
        self.NKV = SKV // 128  # kv chunks of 128
        self.NSB = SQ // 128  # query blocks of 128
        self.SCH = min(512, SQ)  # moving-N chunk for the S^T matmul
        self.NSC = SQ // self.SCH
        self.SHALF = min(512, SQ)  # scatter s-half width (== group width)
        self.scale = float(D) ** -0.5


# ---------------------------------------------------------------------------
# Host-side index preprocessing: invert (query -> kv rows) into
# (kv row -> queries) lists, split duplicate (s,j) pairs into passes.
# Pure index bookkeeping; all values are placed by the device scatter.
# ---------------------------------------------------------------------------


def host_prep_scatter(topk_indices, topk_scores, cfg):
    """Invert (query -> kv rows) into (kv row -> queries), merging duplicate
    (s, j) pairs by summing their scores (the same reduction the reference's
    gather+softmax performs).  Emitted per (kv-chunk, s-half) so the device
    scatter pipeline can release the first half of TS^T early."""
    SQ, TOPK, SKV = cfg.SQ, cfg.TOPK, cfg.SKV
    HALF = cfg.SHALF
    NH = SQ // HALF
    s_arr = np.repeat(np.arange(SQ, dtype=np.int64), TOPK)
    j_arr = topk_indices.reshape(-1).astype(np.int64)
    v_arr = topk_scores.reshape(-1).astype(np.float32)

    sj = s_arr * SKV + j_arr
    uniq, inv = np.unique(sj, return_inverse=True)
    vals = np.zeros(len(uniq), dtype=np.float32)
    np.add.at(vals, inv, v_arr)
    sp = (uniq // SKV).astype(np.int64)
    jp = (uniq % SKV).astype(np.int64)

    idx_list, val_list = [], []
    for hf in range(NH):
        m = (sp >= hf * HALF) & (sp < (hf + 1) * HALF)
        sh, jh, vh = sp[m] - hf * HALF, jp[m], vals[m]
        perm2 = np.argsort(jh, kind="stable")
        jps = jh[perm2]
        ng = np.r_[True, np.diff(jps) != 0]
        gs = np.maximum.accumulate(np.where(ng, np.arange(len(jps)), 0))
        slot_sorted = np.arange(len(jps)) - gs
        slot = np.empty(len(jh), dtype=np.int64)
        slot[perm2] = slot_sorted
        nmax = int(slot.max()) + 1 if len(slot) else 1
        nmax = max(2, (nmax + 1) // 2 * 2)
        idx16 = np.full((cfg.NKV, 128, nmax), -1, dtype=np.int16)
        valsd = np.zeros((cfg.NKV, 128, nmax), dtype=np.float32)
        idx16[jh // 128, jh % 128, slot] = sh.astype(np.int16)
        valsd[jh // 128, jh % 128, slot] = vh
        idx_list.append(idx16)
        val_list.append(valsd)
    return idx_list, val_list


def host_identity():
    return np.eye(128, dtype=np.float32)


# ---------------------------------------------------------------------------
# Program builder
# ---------------------------------------------------------------------------


def build_program(cfg, nmaxs, reps=1):
    nc = bacc.Bacc("TRN2", debug=False)
    HPC, SQ, SKV, D, NKV, NSB = cfg.HPC, cfg.SQ, cfg.SKV, cfg.D, cfg.NKV, cfg.NSB
    SCH, NSC = cfg.SCH, cfg.NSC
    npass = len(nmaxs)

    q = nc.dram_tensor("q", [HPC, SQ, D], FP, kind="ExternalInput").ap()
    k = nc.dram_tensor("k", [HPC, SKV, D], FP, kind="ExternalInput").ap()
    v = nc.dram_tensor("v", [HPC, SKV, D], FP, kind="ExternalInput").ap()
    ident = nc.dram_tensor("ident", [128, 128], FP, kind="ExternalInput").ap()
    sc_idx = [
        nc.dram_tensor(f"sc_idx_{p}", [NKV, 128, nmaxs[p]], I16, kind="ExternalInput").ap()
        for p in range(npass)
    ]
    sc_val = [
        nc.dram_tensor(f"sc_val_{p}", [NKV, 128, nmaxs[p]], FP, kind="ExternalInput").ap()
        for p in range(npass)
    ]
    out = nc.dram_tensor("out", [HPC, SQ, D], FP, kind="ExternalOutput").ap()

    STG = 8  # kv-chunks per staging DMA

    with tile.TileContext(nc) as tc:
        import contextlib

        ctx = contextlib.ExitStack()
        with ctx:
            const_pool = ctx.enter_context(tc.tile_pool(name="const", bufs=1))
            tst_pool = ctx.enter_context(tc.tile_pool(name="tst", bufs=1))
            stage_pool = ctx.enter_context(tc.tile_pool(name="stage", bufs=2))
            ktr_pool = ctx.enter_context(tc.tile_pool(name="ktr", bufs=2))
            et_pool = ctx.enter_context(tc.tile_pool(name="et", bufs=2))
            small_pool = ctx.enter_context(tc.tile_pool(name="small", bufs=2))
            out_pool = ctx.enter_context(tc.tile_pool(name="outp", bufs=3))
            s_psum = ctx.enter_context(tc.tile_pool(name="sps", bufs=2, space="PSUM"))
            tr_psum = ctx.enter_context(tc.tile_pool(name="trps", bufs=2, space="PSUM"))
            o_psum = ctx.enter_context(tc.tile_pool(name="ops", bufs=2, space="PSUM"))

            identity = const_pool.tile([128, 128], FP, tag="ident")
            nc.sync.dma_start(identity[:], ident[:])

            # ---------------- timed body (optionally looped for timing) ----
            def _body(_iv=None):
                _build_body(
                    nc, tc, cfg, nmaxs, q, k, v, sc_idx, sc_val, out, identity,
                    tst_pool, stage_pool, ktr_pool, et_pool, small_pool,
                    out_pool, s_psum, tr_psum, o_psum,
                )

            if reps == 1:
                _body()
            else:
                with tc.For_i(
                    0, reps, 1,
                    hint_engines=(
                        mybir.EngineType.PE,
                        mybir.EngineType.DVE,
                        mybir.EngineType.Activation,
                        mybir.EngineType.Pool,
                        mybir.EngineType.SP,
                    ),
                ):
                    _body()

    nc.compile()
    return nc


def _build_body(nc, tc, cfg, nmaxs, q, k, v, sc_idx, sc_val, out, identity,
                tst_pool, stage_pool, ktr_pool, et_pool, small_pool,
                out_pool, s_psum, tr_psum, o_psum):
    HPC, SQ, SKV, D, NKV, NSB = cfg.HPC, cfg.SQ, cfg.SKV, cfg.D, cfg.NKV, cfg.NSB
    STG = 4          # kv-chunks per staging DMA
    SGRP = cfg.SHALF      # query-group width (= scatter half width)
    NGRP = SQ // SGRP
    NSBG = SGRP // 128    # s-blocks per group

    # ---------------- prep + scatter, ordered for earliest S start ---------
    tst = tst_pool.tile([128, NGRP, NKV, SGRP], BF, tag="tst")
    ktrs, qtrs, vaugs = [], [], []

    def _prep_q(h):
        qtr = ktr_pool.tile([128, SQ], BF, tag="qtr")
        qview = q[h].rearrange("(n p) d -> p n d", p=128)
        for half in range(2):
            qst = stage_pool.tile([128, NSB // 2, D], FP, tag="qst")
            nc.sync.dma_start(
                qst[:], qview[:, half * (NSB // 2) : (half + 1) * (NSB // 2), :]
            )
            for bb in range(0, NSB // 2, 2):
                b = half * (NSB // 2) + bb
                trp = tr_psum.tile([128, 256], FP, tag="trp")
                nc.tensor.transpose(trp[:, 0:128], qst[:, bb, :], identity[:])
                nc.tensor.transpose(trp[:, 128:256], qst[:, bb + 1, :], identity[:])
                nc.vector.tensor_scalar_mul(
                    qtr[:, b * 128 : (b + 2) * 128], trp[:], cfg.scale
                )
        return qtr

    def _prep_k(h):
        ktr = ktr_pool.tile([128, SKV], BF, tag="ktr")
        kview = k[h].rearrange("(n p) d -> p n d", p=128)
        for g in range(NKV // STG):
            kst = stage_pool.tile([128, STG, D], FP, tag="kst")
            nc.sync.dma_start(kst[:], kview[:, g * STG : (g + 1) * STG, :])
            for j in range(0, STG, 2):
                J = g * STG + j
                trp = tr_psum.tile([128, 256], FP, tag="trp")
                nc.tensor.transpose(trp[:, 0:128], kst[:, j, :], identity[:])
                nc.tensor.transpose(trp[:, 128:256], kst[:, j + 1, :], identity[:])
                nc.vector.tensor_copy(ktr[:, J * 128 : (J + 2) * 128], trp[:])
        return ktr

    def _prep_v(h):
        vaug = ktr_pool.tile([128, NKV, D + 1], BF, tag="vaug")
        vview = v[h].rearrange("(n p) d -> p n d", p=128)
        for g in range(NKV // STG):
            vst = stage_pool.tile([128, STG, D], FP, tag="vst")
            nc.sync.dma_start(vst[:], vview[:, g * STG : (g + 1) * STG, :])
            nc.vector.tensor_copy(vaug[:, g * STG : (g + 1) * STG, 0:D], vst[:])
        nc.vector.memset(vaug[:, :, D : D + 1], 1.0)
        return vaug

    qtrs.append(_prep_q(0))
    ktrs.append(_prep_k(0))

    with tc.tile_pool(name="scst", bufs=2) as sc_pool:
        for g in range(NGRP):
            nm = nmaxs[g]
            idx_t = sc_pool.tile([128, NKV, nm], I16, tag="sidx")
            val_t = sc_pool.tile([128, NKV, nm], FP, tag="sval")
            valb_t = sc_pool.tile([128, NKV, nm], BF, tag="svalb")
            nc.sync.dma_start(idx_t[:], sc_idx[g].rearrange("n p m -> p n m"))
            nc.sync.dma_start(val_t[:], sc_val[g].rearrange("n p m -> p n m"))
            nc.vector.tensor_copy(valb_t[:], val_t[:])
            for J in range(NKV):
                nc.gpsimd.local_scatter(
                    tst[:, g, J, :],
                    valb_t[:, J, :], idx_t[:, J, :],
                    channels=128, num_elems=SGRP, num_idxs=nm,
                )

    vaugs.append(_prep_v(0))
    qtrs.append(_prep_q(1))
    ktrs.append(_prep_k(1))
    vaugs.append(_prep_v(1))

    # ---------------- S^T -> exp -> *TS -> O, per (head, s-group) ----------
    for h in range(HPC):
        ktr, qtr, vaug = ktrs[h], qtrs[h], vaugs[h]
        for g in range(NGRP):
            sl = slice(g * SGRP, (g + 1) * SGRP)
            et = et_pool.tile([128, NKV, SGRP], BF, tag="et")
            for J in range(0, NKV, 2):
                sp = s_psum.tile([128, 2, SGRP], FP, tag="sps")
                nc.tensor.matmul(
                    sp[:, 0, :], ktr[:, J * 128 : (J + 1) * 128], qtr[:, sl],
                    start=True, stop=True,
                )
                nc.tensor.matmul(
                    sp[:, 1, :], ktr[:, (J + 1) * 128 : (J + 2) * 128], qtr[:, sl],
                    start=True, stop=True,
                )
                nc.scalar.activation(
                    et[:, J : J + 2, :], sp[:], mybir.ActivationFunctionType.Exp
                )
                nc.vector.tensor_mul(
                    et[:, J : J + 2, :], et[:, J : J + 2, :],
                    tst[:, g, J : J + 2, :],
                )
            for bb in range(NSBG):
                b = g * NSBG + bb
                op = o_psum.tile([128, D + 1], FP, tag="ops")
                for J in range(NKV):
                    nc.tensor.matmul(
                        op[:], et[:, J, bb * 128 : (bb + 1) * 128], vaug[:, J, :],
                        start=(J == 0), stop=(J == NKV - 1),
                    )
                recip = small_pool.tile([128, 1], FP, tag="recip")
                nc.vector.reciprocal(recip[:], op[:, D : D + 1])
                ot = out_pool.tile([128, D], FP, tag="ot")
                nc.vector.tensor_scalar_mul(ot[:], op[:, 0:D], recip[:])
                nc.sync.dma_start(out[h, b * 128 : (b + 1) * 128, :], ot[:])




# ---------------------------------------------------------------------------
# Entry point: full unsharded inputs -> full output.
# Sharding: head-parallel, 2 heads per NeuronCore across 8 cores; the
# topk index/score tensors are shared by all cores.
# ---------------------------------------------------------------------------

_CACHE = {}


def kernel(q, k, v, topk_indices, topk_scores):
    q = np.ascontiguousarray(np.asarray(q), dtype=np.float32)
    k = np.ascontiguousarray(np.asarray(k), dtype=np.float32)
    v = np.ascontiguousarray(np.asarray(v), dtype=np.float32)
    ti = np.asarray(topk_indices)
    ts = np.asarray(topk_scores, dtype=np.float32)
    B, H, SQ, D = q.shape
    SKV = k.shape[2]
    TOPK = ti.shape[-1]
    assert B == 1 and H == 16 and SQ == 1024 and SKV == 4096 and D == 128

    cfg = Cfg(HPC=H // 8, SQ=SQ, SKV=SKV, D=D, TOPK=TOPK)
    idx_arrs, val_arrs = host_prep_scatter(ti[0], ts[0], cfg)
    nmaxs = tuple(a.shape[-1] for a in idx_arrs)

    nc = _CACHE.get(nmaxs)
    if nc is None:
        nc = build_program(cfg, list(nmaxs), reps=1)
        _CACHE[nmaxs] = nc

    from concourse.bass_utils import run_bass_kernel_spmd

    ident = np.eye(128, dtype=np.float32)
    in_maps = []
    for i in range(8):
        m = {
            "q": q[0, 2 * i : 2 * i + 2],
            "k": k[0, 2 * i : 2 * i + 2],
            "v": v[0, 2 * i : 2 * i + 2],
            "ident": ident,
        }
        for p, (ia, va) in enumerate(zip(idx_arrs, val_arrs)):
            m[f"sc_idx_{p}"] = ia
            m[f"sc_val_{p}"] = va
        in_maps.append(m)

    res = run_bass_kernel_spmd(nc, in_maps, list(range(8)))
    out = np.stack([res.results[i]["out"] for i in range(8)])
    return out.reshape(1, H, SQ, D).astype(np.float32)

